# revision 5
# baseline (speedup 1.0000x reference)
"""Trainium2 Bass kernel for nn_Cluster_7017976562037 (vq_codebook).

reference:
  labels = argmin_k ||x_s - c_k||^2          x: [B,S,D] f32, c: [K,D] f32
  mask[b,i,j] = (labels[b,i] == labels[b,j]) as f32
returns (mask [B,S,S] f32, labels [B,S] int32)

Strategy (data-parallel over batch, 8 cores, 4 batches/core):
  per 128-sample tile: PE-transpose x -> xT chunks, fp32 matmul G = x @ cT
  (accumulated over 16 d-chunks in PSUM), ACT computes u = 2G - x_sq,
  DVE adds -c_sq and max-reduces (u2 = -d2 with bit-exact mirrored fp32
  rounding vs the reference), DVE max_index gives argmin labels.
  Mask: broadcast the 2048 labels to all partitions (PE ones-outer-product),
  then tensor_scalar is_equal per 128-row tile, split across DVE/GPSIMD.
"""

import sys

if "/opt/trn_rl_repo" not in sys.path:
    sys.path.insert(0, "/opt/trn_rl_repo")

import numpy as np

B, S, D, K = 32, 2048, 2048, 256
N_CORES = 8
BPC = B // N_CORES          # batches per core
P = 128                     # partitions
ST = S // P                 # 16 s-tiles per batch
DC = D // P                 # 16 contraction chunks

_NC_CACHE = {}


def _build_nc(bpc=BPC, s=S, d=D, k=K):
    import concourse.bass as bass
    import concourse.tile as tile
    from concourse import bacc, mybir
    from concourse.masks import make_identity

    st, dc = s // P, d // P
    f32 = mybir.dt.float32
    i32 = mybir.dt.int32
    u32 = mybir.dt.uint32
    AF = mybir.ActivationFunctionType
    OP = mybir.AluOpType

    nc = bacc.Bacc("TRN2", target_bir_lowering=False, debug=False,
                   num_devices=N_CORES)

    x_dram = nc.dram_tensor("learning_state", [bpc, s, d], f32,
                            kind="ExternalInput")
    ct_dram = nc.dram_tensor("centersT", [d, k], f32, kind="ExternalInput")
    ncsq_dram = nc.dram_tensor("neg_c_sq", [P, k], f32, kind="ExternalInput")
    mask_dram = nc.dram_tensor("state_weight", [bpc, s, s], f32,
                               kind="ExternalOutput")
    lab_dram = nc.dram_tensor("predicted_labels", [bpc, s], i32,
                              kind="ExternalOutput")

    with tile.TileContext(nc) as tc:
        with (
            tc.tile_pool(name="const", bufs=1) as const,
            tc.tile_pool(name="xin", bufs=3) as xin,
            tc.tile_pool(name="sq", bufs=2) as sqp,
            tc.tile_pool(name="xt", bufs=2) as xtp,
            tc.tile_pool(name="small", bufs=6) as small,
            tc.tile_pool(name="lab", bufs=2) as labp,
            tc.tile_pool(name="mout", bufs=4) as mout,
            tc.tile_pool(name="px", bufs=3, space="PSUM") as px,
            tc.tile_pool(name="pg", bufs=2, space="PSUM") as pg,
            tc.tile_pool(name="pl", bufs=2, space="PSUM") as pl,
        ):
            identity = const.tile([P, P], f32)
            make_identity(nc, identity[:])
            ct_sb = const.tile([P, dc, k], f32)
            nc.sync.dma_start(out=ct_sb[:],
                              in_=ct_dram.rearrange("(c p) k -> p c k", p=P))
            ncsq_sb = const.tile([P, k], f32)
            nc.sync.dma_start(out=ncsq_sb[:], in_=ncsq_dram[:])
            ones1 = const.tile([1, P], f32)
            nc.vector.memset(ones1[:], 1.0)

            for b in range(bpc):
                # ---------------- phase 1: labels ----------------
                labels_colf = labp.tile([P, st], f32)
                for t in range(st):
                    x_tile = xin.tile([P, d], f32)
                    nc.sync.dma_start(out=x_tile[:],
                                      in_=x_dram[b, P * t:P * (t + 1), :])
                    # x_sq = sum(x^2) along free dim (ACT accumulate)
                    sq_scr = sqp.tile([P, d], f32)
                    x_sq = small.tile([P, 1], f32)
                    nc.scalar.activation(sq_scr[:], x_tile[:], AF.Square,
                                         accum_out=x_sq[:])
                    neg_x_sq = small.tile([P, 1], f32)
                    nc.vector.tensor_scalar_mul(neg_x_sq[:], x_sq[:], -1.0)
                    # transpose dc chunks of x_tile via PE, evacuate to SBUF
                    xT = xtp.tile([P, dc, P], f32)
                    for g in range(dc // 4):
                        ps_x = px.tile([P, 512], f32)
                        for j in range(4):
                            cidx = 4 * g + j
                            nc.tensor.transpose(
                                ps_x[:, P * j:P * (j + 1)],
                                x_tile[:, P * cidx:P * (cidx + 1)],
                                identity[:])
                        nc.scalar.copy(
                            xT[:, 4 * g:4 * (g + 1), :].rearrange(
                                "p a b -> p (a b)"),
                            ps_x[:])
                    # G = x @ cT, accumulated over chunks in PSUM
                    g_ps = pg.tile([P, k], f32)
                    for ci in range(dc):
                        nc.tensor.matmul(g_ps[:], xT[:, ci, :], ct_sb[:, ci, :],
                                         start=(ci == 0), stop=(ci == dc - 1))
                    # u = fl(2G - x_sq)   (== -(fl(x_sq - 2G)) bit-exactly)
                    u = small.tile([P, k], f32)
                    nc.scalar.activation(u[:], g_ps[:], AF.Identity,
                                         bias=neg_x_sq[:], scale=2.0)
                    # u2 = fl(u - c_sq) = -d2 ; umax = max_k u2
                    u2 = small.tile([P, k], f32)
                    umax = small.tile([P, 1], f32)
                    nc.vector.tensor_tensor(u2[:], u[:], ncsq_sb[:], OP.add)
                    nc.vector.tensor_reduce(umax[:], u2[:],
                                            mybir.AxisListType.X, OP.max)
                    idx8 = small.tile([P, 8], u32)
                    um = umax[:]
                    um8 = bass.AP(um.tensor, um.offset, [um.ap[0], [0, 8]])
                    nc.vector.max_index(idx8[:], um8, u2[:])
                    nc.vector.tensor_copy(labels_colf[:, t:t + 1],
                                          idx8[:, 0:1])

                # ---- labels row, int32 out, broadcast to all partitions ----
                ps_lab = pl.tile([st, P], f32)
                nc.tensor.transpose(ps_lab[:], labels_colf[:], identity[:])
                labT_f = labp.tile([st, P], f32)
                nc.scalar.copy(labT_f[:], ps_lab[:])
                labT_i = labp.tile([st, P], i32)
                nc.vector.tensor_copy(labT_i[:], ps_lab[:])
                nc.sync.dma_start(
                    out=lab_dram[b].rearrange("(t p) -> t p", t=st),
                    in_=labT_i[:])
                labT_row = labp.tile([1, s], f32)
                nc.gpsimd.dma_start(out=labT_row[:], in_=labT_f[:])
                lab_bcast = labp.tile([P, s], f32)
                for jg in range(s // 512):
                    ps_b = px.tile([P, 512], f32, tag="ps_x")
                    nc.tensor.matmul(ps_b[:], ones1[:],
                                     labT_row[0:1, 512 * jg:512 * (jg + 1)],
                                     start=True, stop=True)
                    nc.scalar.copy(lab_bcast[:, 512 * jg:512 * (jg + 1)],
                                   ps_b[:])

                # ---------------- phase 2: mask ----------------
                for t in range(st):
                    m_tile = mout.tile([P, s], f32)
                    eng = nc.vector if t % 2 == 0 else nc.gpsimd
                    eng.tensor_scalar(m_tile[:], lab_bcast[:],
                                      labels_colf[:, t:t + 1], None,
                                      OP.is_equal)
                    nc.sync.dma_start(
                        out=mask_dram[b, P * t:P * (t + 1), :],
                        in_=m_tile[:])
    nc.compile()
    return nc


def _get_nc():
    key = (BPC, S, D, K)
    if key not in _NC_CACHE:
        _NC_CACHE[key] = _build_nc()
    return _NC_CACHE[key]


def _host_prep(centers):
    centersT = np.ascontiguousarray(centers.T).astype(np.float32, copy=False)
    # c_sq computed exactly like the reference (jnp.sum(c*c, axis=1) on cpu)
    try:
        import jax

        with jax.default_device(jax.devices("cpu")[0]):
            import jax.numpy as jnp

            c_sq = np.asarray(jnp.sum(jnp.asarray(centers) *
                                      jnp.asarray(centers), axis=1))
    except Exception:
        c_sq = (centers.astype(np.float32) ** 2).sum(axis=1,
                                                     dtype=np.float32)
    neg_c_sq = np.ascontiguousarray(
        np.repeat((-c_sq.astype(np.float32))[None, :], P, axis=0))
    return centersT, neg_c_sq


def kernel(learning_state, centers):
    from concourse.bass_utils import run_bass_kernel_spmd

    learning_state = np.asarray(learning_state, dtype=np.float32)
    centers = np.asarray(centers, dtype=np.float32)
    nc = _get_nc()
    centersT, neg_c_sq = _host_prep(centers)
    in_maps = [
        {
            "learning_state": learning_state[i * BPC:(i + 1) * BPC],
            "centersT": centersT,
            "neg_c_sq": neg_c_sq,
        }
        for i in range(N_CORES)
    ]
    res = run_bass_kernel_spmd(nc, in_maps, list(range(N_CORES)))
    mask = np.concatenate([res.results[i]["state_weight"]
                           for i in range(N_CORES)], axis=0)
    labels = np.concatenate([res.results[i]["predicted_labels"]
                             for i in range(N_CORES)], axis=0)
    return mask, labels


# revision 7
# speedup vs baseline: 2.5399x; 2.5399x over previous
"""Trainium2 Bass kernel for nn_Cluster_7017976562037 (vq_codebook).

reference:
  labels = argmin_k ||x_s - c_k||^2          x: [B,S,D] f32, c: [K,D] f32
  mask[b,i,j] = (labels[b,i] == labels[b,j]) as f32
returns (mask [B,S,S] f32, labels [B,S] int32)

Data-parallel over batch: 8 cores x 4 batches. Host-side prep (layout
only): x is re-tiled into PE-ready transposed blocks, x_sq/c_sq are
computed with jax-CPU so the device d2 rounding bit-matches the
reference. Device per 128-sample tile: fp32 matmul G = x @ cT
(16 PSUM-accumulated chunk matmuls), ACT computes u = 2G - x_sq, DVE
adds -c_sq (u2 = -d2, exact fp32 mirror of the reference), then DVE
max-reduce + max_index give the argmin labels. The [S,S] mask is
tensor_scalar is_equal per 128-row tile on DVE against a label row
broadcast built by a PE ones-outer-product.
"""

import sys

if "/opt/trn_rl_repo" not in sys.path:
    sys.path.insert(0, "/opt/trn_rl_repo")

import numpy as np

B, S, D, K = 32, 2048, 2048, 256
N_CORES = 8
BPC = B // N_CORES          # batches per core
P = 128                     # partitions
ST = S // P                 # 16 s-tiles per batch
DC = D // P                 # 16 contraction chunks

_NC_CACHE = {}


def _build_nc(bpc=BPC, s=S, d=D, k=K):
    import concourse.bass as bass
    import concourse.tile as tile
    from concourse import bacc, mybir
    from concourse.masks import make_identity

    st, dc = s // P, d // P
    f32 = mybir.dt.float32
    i32 = mybir.dt.int32
    u32 = mybir.dt.uint32
    AF = mybir.ActivationFunctionType
    OP = mybir.AluOpType

    nc = bacc.Bacc("TRN2", target_bir_lowering=False, debug=False,
                   num_devices=N_CORES)

    # xT blocks: [bpc, st, P, dc, P] where [t, p, c, m] = x[b, 128t+m, 128c+p]
    xt_dram = nc.dram_tensor("xt", [bpc, st, P, dc, P], f32,
                             kind="ExternalInput")
    ct_dram = nc.dram_tensor("centersT", [d, k], f32, kind="ExternalInput")
    ncsq_dram = nc.dram_tensor("neg_c_sq", [P, k], f32, kind="ExternalInput")
    # neg_x_sq tiled: [bpc, P, st] where [p, t] = -x_sq[b, 128t+p]
    nxsq_dram = nc.dram_tensor("neg_x_sq", [bpc, P, st], f32,
                               kind="ExternalInput")
    mask_dram = nc.dram_tensor("state_weight", [bpc, s, s], f32,
                               kind="ExternalOutput")
    lab_dram = nc.dram_tensor("predicted_labels", [bpc, s], i32,
                              kind="ExternalOutput")

    with tile.TileContext(nc) as tc:
        with (
            tc.tile_pool(name="const", bufs=1) as const,
            tc.tile_pool(name="xt", bufs=3) as xtp,
            tc.tile_pool(name="small", bufs=6) as small,
            tc.tile_pool(name="lab", bufs=2) as labp,
            tc.tile_pool(name="mout", bufs=4) as mout,
            tc.tile_pool(name="px", bufs=2, space="PSUM") as px,
            tc.tile_pool(name="pg", bufs=3, space="PSUM") as pg,
            tc.tile_pool(name="pl", bufs=2, space="PSUM") as pl,
        ):
            identity = const.tile([P, P], f32)
            make_identity(nc, identity[:])
            ct_sb = const.tile([P, dc, k], f32)
            nc.sync.dma_start(out=ct_sb[:],
                              in_=ct_dram.rearrange("(c p) k -> p c k", p=P))
            ncsq_sb = const.tile([P, k], f32)
            nc.sync.dma_start(out=ncsq_sb[:], in_=ncsq_dram[:])
            ones1 = const.tile([1, P], f32)
            nc.vector.memset(ones1[:], 1.0)

            for b in range(bpc):
                # ---------------- phase 1: labels ----------------
                nxsq = labp.tile([P, st], f32)
                nc.sync.dma_start(out=nxsq[:], in_=nxsq_dram[b])
                labels_colf = labp.tile([P, st], f32)
                for t in range(st):
                    xT = xtp.tile([P, dc, P], f32)
                    nc.sync.dma_start(out=xT[:], in_=xt_dram[b, t])
                    # G = x @ cT, accumulated over chunks in PSUM
                    g_ps = pg.tile([P, k], f32)
                    for ci in range(dc):
                        nc.tensor.matmul(g_ps[:], xT[:, ci, :], ct_sb[:, ci, :],
                                         start=(ci == 0), stop=(ci == dc - 1))
                    # u = fl(2G - x_sq)   (== -(fl(x_sq - 2G)) bit-exactly)
                    u = small.tile([P, k], f32)
                    nc.scalar.activation(u[:], g_ps[:], AF.Identity,
                                         bias=nxsq[:, t:t + 1], scale=2.0)
                    # u2 = fl(u - c_sq) = -d2 ; umax = max_k u2
                    u2 = small.tile([P, k], f32)
                    umax = small.tile([P, 1], f32)
                    nc.vector.tensor_tensor(u2[:], u[:], ncsq_sb[:], OP.add)
                    nc.vector.tensor_reduce(umax[:], u2[:],
                                            mybir.AxisListType.X, OP.max)
                    idx8 = small.tile([P, 8], u32)
                    um = umax[:]
                    um8 = bass.AP(um.tensor, um.offset, [um.ap[0], [0, 8]])
                    nc.vector.max_index(idx8[:], um8, u2[:])
                    nc.vector.tensor_copy(labels_colf[:, t:t + 1],
                                          idx8[:, 0:1])

                # ---- labels row, int32 out, broadcast to all partitions ----
                ps_lab = pl.tile([st, P], f32)
                nc.tensor.transpose(ps_lab[:], labels_colf[:], identity[:])
                labT_f = labp.tile([st, P], f32)
                nc.scalar.copy(labT_f[:], ps_lab[:])
                labT_i = labp.tile([st, P], i32)
                nc.vector.tensor_copy(labT_i[:], ps_lab[:])
                nc.sync.dma_start(
                    out=lab_dram[b].rearrange("(t p) -> t p", t=st),
                    in_=labT_i[:])
                labT_row = labp.tile([1, s], f32)
                nc.gpsimd.dma_start(out=labT_row[:], in_=labT_f[:])
                lab_bcast = labp.tile([P, s], f32)
                for jg in range(s // 512):
                    ps_b = px.tile([P, 512], f32, tag="ps_b")
                    nc.tensor.matmul(ps_b[:], ones1[:],
                                     labT_row[0:1, 512 * jg:512 * (jg + 1)],
                                     start=True, stop=True)
                    nc.scalar.copy(lab_bcast[:, 512 * jg:512 * (jg + 1)],
                                   ps_b[:])

                # ---------------- phase 2: mask ----------------
                for t in range(st):
                    m_tile = mout.tile([P, s], f32)
                    nc.vector.tensor_scalar(m_tile[:], lab_bcast[:],
                                            labels_colf[:, t:t + 1], None,
                                            OP.is_equal)
                    nc.sync.dma_start(
                        out=mask_dram[b, P * t:P * (t + 1), :],
                        in_=m_tile[:])
    nc.compile()
    return nc


def _get_nc():
    key = (BPC, S, D, K)
    if key not in _NC_CACHE:
        _NC_CACHE[key] = _build_nc()
    return _NC_CACHE[key]


def _sum_sq_rows(a):
    """Row-wise sum of squares, matching the reference's jnp.sum(x*x, -1)
    bit-for-bit (jax CPU); numpy fallback if jax-cpu is unavailable."""
    try:
        import jax

        with jax.default_device(jax.devices("cpu")[0]):
            import jax.numpy as jnp

            ja = jnp.asarray(a)
            return np.asarray(jnp.sum(ja * ja, axis=-1))
    except Exception:
        return (a.astype(np.float32) ** 2).sum(axis=-1, dtype=np.float32)


def _prep_centers(centers):
    centersT = np.ascontiguousarray(centers.T).astype(np.float32, copy=False)
    c_sq = _sum_sq_rows(centers)
    neg_c_sq = np.ascontiguousarray(
        np.repeat((-c_sq.astype(np.float32))[None, :], P, axis=0))
    return centersT, neg_c_sq


def _prep_x(x):
    """x [b, s, d] -> (xt blocks [b, st, P, dc, P], neg_x_sq [b, P, st])."""
    b, s, d = x.shape
    st, dc = s // P, d // P
    xt = np.ascontiguousarray(
        x.reshape(b, st, P, dc, P).transpose(0, 1, 4, 3, 2))
    x_sq = _sum_sq_rows(x)                       # [b, s]
    nxsq = np.ascontiguousarray(
        (-x_sq.astype(np.float32)).reshape(b, st, P).transpose(0, 2, 1))
    return xt, nxsq


def kernel(learning_state, centers):
    from concourse.bass_utils import run_bass_kernel_spmd

    learning_state = np.asarray(learning_state, dtype=np.float32)
    centers = np.asarray(centers, dtype=np.float32)
    nc = _get_nc()
    centersT, neg_c_sq = _prep_centers(centers)
    in_maps = []
    for i in range(N_CORES):
        xs = learning_state[i * BPC:(i + 1) * BPC]
        xt, nxsq = _prep_x(xs)
        in_maps.append({
            "xt": xt,
            "centersT": centersT,
            "neg_c_sq": neg_c_sq,
            "neg_x_sq": nxsq,
        })
    res = run_bass_kernel_spmd(nc, in_maps, list(range(N_CORES)))
    mask = np.concatenate([res.results[i]["state_weight"]
                           for i in range(N_CORES)], axis=0)
    labels = np.concatenate([res.results[i]["predicted_labels"]
                             for i in range(N_CORES)], axis=0)
    return mask, labels
